# revision 1
# baseline (speedup 1.0000x reference)
"""Trainium2 Bass kernel for CenterGeoAttention (N=65536, D=1024, H=16).

Strategy (row-shard N across 8 cores, activations kept feature-major):

Host algebra reduces the attention almost entirely:
  - q = LN(h[c]) @ Wq is input-only -> fold into Wkp = (Wk @ Qblockdiag) * gamma_a
    (1024x16), so logits need no K projection matmul.
  - LN is folded into rank-1 corrections around raw-h matmuls (means/rstd are
    per-row column scalings that commute with the feature-contraction).
  - The weighted V sum never materializes V: G = (p*r)^T @ h (16x1024 per core),
    AllReduce-add [G | PRM | S], then out_center = blockdiag(G_hat @ Wv),
    h_c_new = h[c] + 0.5 * Wo^T @ out_center.
  - h_cat @ W = h @ W_top + rank-1(h_c_new @ W_bot) splits the 2D-wide MLP/gate
    matmuls in half.
Device per core: 3 big fp32r matmuls (h@W1t, h@Wgt, silu@W2) of 8192x1024x1024
plus the cheap attention pass and one 66KB AllReduce.
"""

import os
import ml_dtypes
import numpy as np

import concourse.bass as bass
import concourse.bacc as bacc
import concourse.tile as tile
import concourse.mybir as mybir
from concourse.bass_utils import run_bass_kernel_spmd

F32 = mybir.dt.float32
F32R = mybir.dt.float32r
BF16 = mybir.dt.bfloat16
AF = mybir.ActivationFunctionType
OP = mybir.AluOpType
AX = mybir.AxisListType

NCORES = 8
N, D, H, HD, BIAS = 65536, 1024, 16, 64, 128
NS = N // NCORES            # 8192 rows per core
CH = 512                    # row-chunk
NCH = NS // CH              # 16 chunks
KT = D // 128               # 8 feature tiles
EPS = 1e-5
RES = 0.5

_CACHE = {}
LAST_RESULTS = None  # BassKernelResults from the most recent run (for test.py)


def _build(ncores=NCORES, variant="full", nch=NCH, stage=99):
    nc = bacc.Bacc("TRN2", target_bir_lowering=False, debug=False,
                   num_devices=ncores)

    def din(name, shape, dt=F32R):
        return nc.dram_tensor(name, list(shape), dt, kind="ExternalInput").ap()

    # per-core tensors
    hT = din("hT", (D, NS))               # h_shard^T
    hN = din("hN", (NS, D), BF16)         # h_shard natural (bf16)
    bT = din("bT", (BIAS, NS), BF16)      # bias_feat^T shard (bf16)
    # shared weights
    Wkp = din("Wkp", (D, H))
    Wb = din("Wb", (BIAS, H), BF16)
    W1t = din("W1t", (D, D))
    Wgt = din("Wgt", (D, D))
    W2h = din("W2h", (D, D))
    Wv = din("Wv", (D, D), BF16)
    Wo = din("Wo", (D, D), BF16)
    W1b = din("W1b", (D, D), BF16)
    Wgb = din("Wgb", (D, D), BF16)
    # small constants
    idn = din("idn", (128, 128), F32)
    ones128 = din("ones128", (128, 1), F32R)
    ncg = din("ncg", (H, 1), F32)         # -cg per head
    cbv = din("cbv", (H, 1), F32)         # cb per head (exp bias)
    gb16 = din("gb16", (H, D), F32)       # gamma_a broadcast rows
    bb16 = din("bb16", (H, D), F32)       # beta_a broadcast rows
    hcv = din("hcv", (128, KT), F32)      # h[c] as [p, m]
    b1v = din("b1v", (128, KT), F32)
    bgv = din("bgv", (128, KT), F32)
    b2v = din("b2v", (128, KT), F32)      # 0.5*b2
    epsv = din("epsv", (1, 1), F32)

    outT = nc.dram_tensor("outT", [D, NS], F32, kind="ExternalOutput").ap()
    outC = nc.dram_tensor("outC", [128, KT], F32, kind="ExternalOutput").ap()

    with tile.TileContext(nc) as tc:
        with (
            tc.tile_pool(name="persist", bufs=1) as pp,
            tc.tile_pool(name="dram", bufs=1, space="DRAM") as dram,
        ):
            # ---- long-lived small tiles ----
            idn_s = pp.tile([128, 128], F32, tag="idn")
            nc.sync.dma_start(out=idn_s[:], in_=idn[:])
            ones_s = pp.tile([128, 1], F32R, tag="ones128")
            nc.sync.dma_start(out=ones_s[:], in_=ones128[:])
            ncg_s = pp.tile([H, 1], F32, tag="ncg")
            nc.sync.dma_start(out=ncg_s[:], in_=ncg[:])
            cbv_s = pp.tile([H, 1], F32, tag="cbv")
            nc.sync.dma_start(out=cbv_s[:], in_=cbv[:])
            hcv_s = pp.tile([128, KT], F32, tag="hcv")
            nc.sync.dma_start(out=hcv_s[:], in_=hcv[:])
            b1v_s = pp.tile([128, KT], F32, tag="b1v")
            nc.sync.dma_start(out=b1v_s[:], in_=b1v[:])
            bgv_s = pp.tile([128, KT], F32, tag="bgv")
            nc.sync.dma_start(out=bgv_s[:], in_=bgv[:])
            b2v_s = pp.tile([128, KT], F32, tag="b2v")
            nc.sync.dma_start(out=b2v_s[:], in_=b2v[:])
            Wkp_s = pp.tile([128, KT * H], F32R, tag="Wkp")
            for k in range(KT):
                nc.sync.dma_start(out=Wkp_s[:, k * H:(k + 1) * H],
                                  in_=Wkp[k * 128:(k + 1) * 128, :])
            Wb_s = pp.tile([BIAS, H], BF16, tag="Wb")
            nc.sync.dma_start(out=Wb_s[:], in_=Wb[:])
            epsv_s = pp.tile([1, 1], F32, tag="epsv")
            nc.sync.dma_start(out=epsv_s[:], in_=epsv[:])

            Gacc = pp.tile([H, D], F32, tag="Gacc")
            sCols = pp.tile([H, NCH], F32, tag="sCols")
            prmCols = pp.tile([H, NCH], F32, tag="prmCols")
            hcn_sb = pp.tile([128, KT], F32, tag="hcn")
            g0_s = pp.tile([128, KT], F32, tag="g0")
            a0_s = pp.tile([128, KT], F32, tag="a0")

            # resident pass-2 weights: loaded during pass 1
            wres_cm = tc.tile_pool(name="p2w", bufs=1)
            wres = wres_cm.__enter__()
            W1t_s = wres.tile([128, KT * D], F32R, tag="W1t")
            Wgt_s = wres.tile([128, KT * D], F32R, tag="Wgt")

            # =========================== PASS 1 ===========================
            psG_cm = tc.tile_pool(name="p1psG", bufs=1, space="PSUM")
            psG = psG_cm.__enter__()
            G = psG.tile([H, D], F32, tag="G")
            with (
                tc.tile_pool(name="p1sb", bufs=1) as sb1,
                tc.tile_pool(name="p1sb2", bufs=2) as sb2,
                tc.tile_pool(name="p1ps", bufs=1, space="PSUM") as ps1,
            ):
                for c in range(nch):
                    c0 = c * CH
                    if c == 2:
                        for k in range(KT):
                            nc.sync.dma_start(
                                out=W1t_s[:, k * D:(k + 1) * D],
                                in_=W1t[k * 128:(k + 1) * 128, :])
                            nc.sync.dma_start(
                                out=Wgt_s[:, k * D:(k + 1) * D],
                                in_=Wgt[k * 128:(k + 1) * 128, :])
                    hTc = sb2.tile([128, KT * CH], F32R, tag="hTc")
                    for k in range(KT):
                        nc.sync.dma_start(
                            out=hTc[:, k * CH:(k + 1) * CH],
                            in_=hT[k * 128:(k + 1) * 128, c0:c0 + CH])
                    hNc = sb2.tile([128, 4 * D], BF16, tag="hNc")
                    for j in range(4):
                        nc.sync.dma_start(
                            out=hNc[:, j * D:(j + 1) * D],
                            in_=hN[c0 + j * 128:c0 + (j + 1) * 128, :])
                    bTc = sb2.tile([BIAS, CH], BF16, tag="bTc")
                    nc.sync.dma_start(out=bTc[:], in_=bT[:, c0:c0 + CH])

                    if stage == 0:
                        ot0 = sb1.tile([128, CH], F32, tag="ot0")
                        nc.vector.tensor_copy(ot0[:], hTc[:, 0:CH].bitcast(F32))
                        nc.sync.dma_start(out=outT[0:128, c0:c0 + CH], in_=ot0[:])
                        continue
                    # row stats: sum(h), sum(h^2) via ones-matmuls
                    sq = sb1.tile([128, KT * CH], F32R, tag="sq")
                    nc.vector.tensor_mul(sq[:], hTc[:], hTc[:])
                    stats_m = ps1.tile([1, CH], F32, tag="stats_m")
                    for k in range(KT):
                        nc.tensor.matmul(stats_m[:], ones_s[:],
                                         hTc[:, k * CH:(k + 1) * CH],
                                         start=(k == 0), stop=(k == KT - 1))
                    stats_q = ps1.tile([1, CH], F32, tag="stats_q")
                    for k in range(KT):
                        nc.tensor.matmul(stats_q[:], ones_s[:],
                                         sq[:, k * CH:(k + 1) * CH],
                                         start=(k == 0), stop=(k == KT - 1))
                    tm = sb2.tile([1, CH], F32, tag="tm")
                    nc.vector.tensor_scalar_mul(tm[:], stats_m[:], 1.0 / D)
                    msq = sb2.tile([1, CH], F32, tag="msq")
                    nc.scalar.square(msq[:], tm[:])
                    var = sb2.tile([1, CH], F32, tag="var")
                    nc.vector.scalar_tensor_tensor(
                        var[:], stats_q[:], 1.0 / D, msq[:],
                        op0=OP.mult, op1=OP.subtract)
                    sd = sb2.tile([1, CH], F32, tag="sd")
                    nc.scalar.activation(sd[:], var[:], AF.Sqrt, bias=epsv_s[:, 0:1])
                    r_t = sb2.tile([1, CH], F32, tag="rt")
                    nc.vector.reciprocal(r_t[:], sd[:])
                    mr_t = sb2.tile([1, CH], F32, tag="mrt")
                    nc.vector.tensor_mul(mr_t[:], tm[:], r_t[:])

                    if stage == 1:
                        ot1 = sb1.tile([1, 2 * CH], F32, tag="ot1")
                        nc.vector.tensor_copy(ot1[:, 0:CH], r_t[:])
                        nc.vector.tensor_copy(ot1[:, CH:2 * CH], mr_t[:])
                        nc.sync.dma_start(out=outT[0:1, c0:c0 + 2 * CH], in_=ot1[:])
                        continue
                    # broadcast r and m*r to 16 partitions
                    rb16 = sb2.tile([H, CH], F32, tag="rb16")
                    nc.gpsimd.partition_broadcast(rb16[:], r_t[:])
                    mrb16 = sb2.tile([H, CH], F32, tag="mrb16")
                    nc.gpsimd.partition_broadcast(mrb16[:], mr_t[:])

                    L = ps1.tile([H, CH], F32, tag="L")
                    for k in range(KT):
                        nc.tensor.matmul(L[:], Wkp_s[:, k * H:(k + 1) * H],
                                         hTc[:, k * CH:(k + 1) * CH],
                                         start=(k == 0), stop=(k == KT - 1))
                    L2 = ps1.tile([H, CH], F32, tag="L2")
                    nc.tensor.matmul(L2[:], Wb_s[:], bTc[:],
                                     start=True, stop=True)
                    t3 = sb1.tile([H, CH], F32, tag="t3")
                    nc.vector.tensor_mul(t3[:], L[:], rb16[:])
                    t4 = sb2.tile([H, CH], F32, tag="t4")
                    nc.vector.scalar_tensor_tensor(
                        t4[:], mrb16[:], ncg_s[:, 0:1], t3[:],
                        op0=OP.mult, op1=OP.add)
                    t5 = sb2.tile([H, CH], F32, tag="t5")
                    nc.vector.tensor_add(t5[:], t4[:], L2[:])
                    if stage == 2:
                        nc.sync.dma_start(out=outT[0:H, c0:c0 + CH], in_=t5[:])
                        continue
                    pT = sb2.tile([H, CH], F32, tag="pT")
                    if stage == 30:
                        nc.scalar.activation(pT[:], t5[:], AF.Exp,
                                             bias=cbv_s[:, 0:1])
                        nc.sync.dma_start(out=outT[0:H, c0:c0 + CH], in_=pT[:])
                        continue
                    nc.scalar.activation(pT[:], t5[:], AF.Exp,
                                         bias=cbv_s[:, 0:1],
                                         accum_out=sCols[:, c:c + 1])
                    if stage == 31:
                        nc.sync.dma_start(out=outT[0:H, c0:c0 + CH], in_=pT[:])
                        continue
                    prT = sb2.tile([H, CH], F32, tag="prT")
                    nc.vector.tensor_mul(prT[:], pT[:], rb16[:])
                    prm_scr = sb1.tile([H, CH], F32, tag="prmscr")
                    nc.vector.tensor_mul(prm_scr[:], pT[:], mrb16[:])
                    nc.vector.reduce_sum(prmCols[:, c:c + 1], prm_scr[:],
                                         axis=AX.X)
                    if stage == 32:
                        nc.sync.dma_start(out=outT[0:H, c0:c0 + CH], in_=prT[:])
                        continue

                    if stage == 3:
                        nc.sync.dma_start(out=outT[0:H, c0:c0 + CH], in_=pT[:])
                        continue
                    # transpose p*r to row-major and accumulate G
                    tp = ps1.tile([128, 4 * H], F32, tag="tp")
                    for j in range(4):
                        nc.tensor.transpose(
                            tp[:, j * H:(j + 1) * H],
                            prT[:, j * 128:(j + 1) * 128],
                            idn_s[0:16, 0:16])
                    pr_nat = sb2.tile([128, 4 * H], BF16, tag="prnat")
                    nc.vector.tensor_copy(pr_nat[:], tp[:])
                    for half in range(2):
                        for j in range(4):
                            nc.tensor.matmul(
                                G[:, half * CH:(half + 1) * CH],
                                pr_nat[:, j * H:(j + 1) * H],
                                hNc[:, j * D + half * CH:j * D + (half + 1) * CH],
                                start=(c == 0 and j == 0),
                                stop=(c == nch - 1 and j == 3))
                nc.vector.tensor_copy(Gacc[:], G[:])
                if variant == "p1" and stage >= 4:
                    nc.sync.dma_start(out=outT[0:H, 0:D], in_=Gacc[:])
                    nc.sync.dma_start(out=outT[H:2 * H, 0:NCH], in_=sCols[:])
                    nc.sync.dma_start(out=outT[2 * H:3 * H, 0:NCH], in_=prmCols[:])

            if variant != "p1":
                psG_cm.__exit__(None, None, None)
            # ---- local partials -> AllReduce ----
                S16 = pp.tile([H, 1], F32, tag="S16")
                nc.vector.reduce_sum(S16[:], sCols[:], axis=AX.X)
                PRM16 = pp.tile([H, 1], F32, tag="PRM16")
                nc.vector.reduce_sum(PRM16[:], prmCols[:], axis=AX.X)

                arin = dram.tile([H, D + 2], F32, tag="arin")
                arout = dram.tile([H, D + 2], F32, tag="arout")
                nc.sync.dma_start(out=arin[:, 0:D], in_=Gacc[:])
                nc.sync.dma_start(out=arin[:, D:D + 1], in_=PRM16[:])
                nc.sync.dma_start(out=arin[:, D + 1:D + 2], in_=S16[:])
                if variant == "nocc":
                    nc.sync.dma_start(out=arout[:], in_=arin[:])
                else:
                    nc.gpsimd.collective_compute(
                        "AllReduce", OP.add,
                        replica_groups=[list(range(ncores))],
                        ins=[arin.opt()], outs=[arout.opt()])
                # ---- G corrections + normalize ----
                with (
                    tc.tile_pool(name="wstream", bufs=2) as ws,
                    tc.tile_pool(name="postsb", bufs=1) as psb,
                    tc.tile_pool(name="postps", bufs=1, space="PSUM") as ps2,
                ):
                    gb16_s = psb.tile([H, D], F32, tag="gb16")
                    nc.sync.dma_start(out=gb16_s[:], in_=gb16[:])
                    bb16_s = psb.tile([H, D], F32, tag="bb16")
                    nc.sync.dma_start(out=bb16_s[:], in_=bb16[:])
                    Gar = psb.tile([H, D], F32, tag="Gar")
                    nc.sync.dma_start(out=Gar[:], in_=arout[:, 0:D])
                    PSar = psb.tile([H, 2], F32, tag="PSar")
                    nc.sync.dma_start(out=PSar[:], in_=arout[:, D:D + 2])
                    Gn = psb.tile([H, D], F32, tag="Gn")
                    nc.vector.tensor_scalar_sub(Gn[:], Gar[:], PSar[:, 0:1])
                    nc.vector.tensor_mul(Gn[:], Gn[:], gb16_s[:])
                    nc.vector.scalar_tensor_tensor(
                        Gn[:], bb16_s[:], PSar[:, 1:2], Gn[:],
                        op0=OP.mult, op1=OP.add)
                    sr = psb.tile([H, 1], F32, tag="sr")
                    nc.vector.reciprocal(sr[:], PSar[:, 1:2])
                    nc.vector.tensor_scalar_mul(Gn[:], Gn[:], sr[:, 0:1])

                    tpg = ps2.tile([128, KT * H], F32, tag="tpg")
                    for m in range(KT):
                        nc.tensor.transpose(
                            tpg[:, m * H:(m + 1) * H],
                            Gn[:, m * 128:(m + 1) * 128],
                            idn_s[0:16, 0:16])
                    GnT = pp.tile([128, KT * H], BF16, tag="GnT")
                    nc.vector.tensor_copy(GnT[:], tpg[:])

                    Wv_s = ws.tile([128, KT * D], BF16, tag="wstream")
                    for k in range(KT):
                        nc.sync.dma_start(out=Wv_s[:, k * D:(k + 1) * D],
                                          in_=Wv[k * 128:(k + 1) * 128, :])
                    OCp = ps2.tile([128, KT * H], F32, tag="OCp")
                    for m in range(KT):
                        for k in range(KT):
                            nc.tensor.matmul(
                                OCp[:, m * H:(m + 1) * H],
                                Wv_s[:, k * D + m * 128:k * D + (m + 1) * 128],
                                GnT[:, k * H:(k + 1) * H],
                                start=(k == 0), stop=(k == KT - 1))
                    ocv = pp.tile([128, KT], BF16, tag="ocv")
                    for m in range(KT):
                        nc.vector.tensor_copy(
                            ocv[0:64, m:m + 1],
                            OCp[0:64, m * H + 2 * m:m * H + 2 * m + 1])
                        nc.vector.tensor_copy(
                            ocv[64:128, m:m + 1],
                            OCp[64:128, m * H + 2 * m + 1:m * H + 2 * m + 2])

                    Wo_s = ws.tile([128, KT * D], BF16, tag="wstream")
                    for k in range(KT):
                        nc.sync.dma_start(out=Wo_s[:, k * D:(k + 1) * D],
                                          in_=Wo[k * 128:(k + 1) * 128, :])
                    hcp = ps2.tile([128, KT], F32, tag="hcp")
                    for m in range(KT):
                        for k in range(KT):
                            nc.tensor.matmul(
                                hcp[:, m:m + 1],
                                Wo_s[:, k * D + m * 128:k * D + (m + 1) * 128],
                                ocv[:, k:k + 1],
                                start=(k == 0), stop=(k == KT - 1))
                    nc.vector.scalar_tensor_tensor(
                        hcn_sb[:], hcp[:], RES, hcv_s[:],
                        op0=OP.mult, op1=OP.add)
                    nc.sync.dma_start(out=outC[:], in_=hcn_sb[:])
                    hcn_bf = pp.tile([128, KT], BF16, tag="hcnbf")
                    nc.vector.tensor_copy(hcn_bf[:], hcn_sb[:])

                    Wgb_s = ws.tile([128, KT * D], BF16, tag="wstream")
                    for k in range(KT):
                        nc.sync.dma_start(out=Wgb_s[:, k * D:(k + 1) * D],
                                          in_=Wgb[k * 128:(k + 1) * 128, :])
                    g0p = ps2.tile([128, KT], F32, tag="g0p")
                    for m in range(KT):
                        for k in range(KT):
                            nc.tensor.matmul(
                                g0p[:, m:m + 1],
                                Wgb_s[:, k * D + m * 128:k * D + (m + 1) * 128],
                                hcn_bf[:, k:k + 1],
                                start=(k == 0), stop=(k == KT - 1))
                    nc.vector.tensor_add(g0_s[:], g0p[:], bgv_s[:])

                    W1b_s = ws.tile([128, KT * D], BF16, tag="wstream")
                    for k in range(KT):
                        nc.sync.dma_start(out=W1b_s[:, k * D:(k + 1) * D],
                                          in_=W1b[k * 128:(k + 1) * 128, :])
                    a0p = ps2.tile([128, KT], F32, tag="a0p")
                    for m in range(KT):
                        for k in range(KT):
                            nc.tensor.matmul(
                                a0p[:, m:m + 1],
                                W1b_s[:, k * D + m * 128:k * D + (m + 1) * 128],
                                hcn_bf[:, k:k + 1],
                                start=(k == 0), stop=(k == KT - 1))
                    nc.vector.tensor_add(a0_s[:], a0p[:], b1v_s[:])

            if variant in ("full", "nocc"):
                # =========================== PASS 2 ===========================
                with (
                    tc.tile_pool(name="p2sb", bufs=2) as sb3,
                    tc.tile_pool(name="p2st", bufs=3) as sb4,
                    tc.tile_pool(name="p2w2", bufs=1) as wres2,
                    tc.tile_pool(name="p2ps", bufs=2, space="PSUM") as ps3,
                ):
                    W2h_s = wres2.tile([128, KT * D], F32R, tag="W2h")
                    for k in range(KT):
                        nc.sync.dma_start(out=W2h_s[:, k * D:(k + 1) * D],
                                          in_=W2h[k * 128:(k + 1) * 128, :])
                    for c in range(NCH):
                        c0 = c * CH
                        hTc = sb3.tile([128, KT * CH], F32R, tag="hTc2")
                        for k in range(KT):
                            nc.sync.dma_start(
                                out=hTc[:, k * CH:(k + 1) * CH],
                                in_=hT[k * 128:(k + 1) * 128, c0:c0 + CH])
                        B = sb3.tile([128, KT * CH], F32R, tag="B")
                        for m in range(KT):
                            A = ps3.tile([128, CH], F32, tag="A")
                            for k in range(KT):
                                nc.tensor.matmul(
                                    A[:], W1t_s[:, k * D + m * 128:k * D + (m + 1) * 128],
                                    hTc[:, k * CH:(k + 1) * CH],
                                    start=(k == 0), stop=(k == KT - 1))
                            nc.scalar.activation(B[:, m * CH:(m + 1) * CH], A[:],
                                                 AF.Silu, bias=a0_s[:, m:m + 1])
                        for m in range(KT):
                            Gt = ps3.tile([128, CH], F32, tag="Gt")
                            for k in range(KT):
                                nc.tensor.matmul(
                                    Gt[:], Wgt_s[:, k * D + m * 128:k * D + (m + 1) * 128],
                                    hTc[:, k * CH:(k + 1) * CH],
                                    start=(k == 0), stop=(k == KT - 1))
                            gs = sb4.tile([128, CH], F32, tag="gs")
                            nc.scalar.activation(gs[:], Gt[:], AF.Sigmoid,
                                                 bias=g0_s[:, m:m + 1])
                            Cp = ps3.tile([128, CH], F32, tag="Cp")
                            for k in range(KT):
                                nc.tensor.matmul(
                                    Cp[:], W2h_s[:, k * D + m * 128:k * D + (m + 1) * 128],
                                    B[:, k * CH:(k + 1) * CH],
                                    start=(k == 0), stop=(k == KT - 1))
                            t6 = sb4.tile([128, CH], F32, tag="t6")
                            nc.vector.scalar_tensor_tensor(
                                t6[:], Cp[:], b2v_s[:, m:m + 1], gs[:],
                                op0=OP.add, op1=OP.mult)
                            ot = sb4.tile([128, CH], F32, tag="ot")
                            nc.vector.tensor_add(
                                ot[:], t6[:],
                                hTc[:, m * CH:(m + 1) * CH].bitcast(F32))
                            nc.sync.dma_start(
                                out=outT[m * 128:(m + 1) * 128, c0:c0 + CH],
                                in_=ot[:])
            wres_cm.__exit__(None, None, None)
    nc.compile()
    return nc


def _get_nc():
    if "nc" not in _CACHE:
        _CACHE["nc"] = _build()
    return _CACHE["nc"]


def kernel(h, center_idx, rbf_ic, seqsep_ic, nbr_idx, local_bias,
           gamma_c, beta_c, gamma_a, beta_a,
           Wq, Wk, Wv, Wo, Wb, W1, b1, W2, b2, Wg, bg):
    global LAST_RESULTS
    f = np.float32
    h = np.asarray(h, f)
    c = int(center_idx)
    rbf_ic = np.asarray(rbf_ic, f)
    seqsep_ic = np.asarray(seqsep_ic, f)
    nbr_idx = np.asarray(nbr_idx)
    local_bias = np.asarray(local_bias, f)
    gamma_c = np.asarray(gamma_c, np.float64)
    beta_c = np.asarray(beta_c, np.float64)
    gamma_a = np.asarray(gamma_a, np.float64)
    beta_a = np.asarray(beta_a, np.float64)
    Wq = np.asarray(Wq, f); Wk = np.asarray(Wk, f); Wv = np.asarray(Wv, f)
    Wo = np.asarray(Wo, f); Wb = np.asarray(Wb, f)
    W1 = np.asarray(W1, f); b1 = np.asarray(b1, f)
    W2 = np.asarray(W2, f); b2 = np.asarray(b2, f)
    Wg = np.asarray(Wg, f); bg = np.asarray(bg, f)

    # ---- host algebra (tiny, no big matmuls) ----
    hc = h[c].astype(np.float64)
    hcl = (hc - hc.mean()) / np.sqrt(hc.var() + EPS) * gamma_c + beta_c
    q = (hcl @ Wq.astype(np.float64)).reshape(H, HD)
    Qm = np.zeros((D, H), np.float64)
    for hh in range(H):
        Qm[hh * HD:(hh + 1) * HD, hh] = q[hh] / np.sqrt(HD)
    Wk1 = Wk.astype(np.float64) @ Qm                    # (D, 16)
    Wkp = (Wk1 * gamma_a[:, None]).astype(f)
    ncg = (-(Wk1 * gamma_a[:, None]).sum(0)).astype(f).reshape(H, 1)
    cbv = (Wk1 * beta_a[:, None]).sum(0).astype(f).reshape(H, 1)

    full_bias = np.zeros((N, local_bias.shape[1]), f)
    full_bias[nbr_idx] = local_bias
    bias_featT = np.ascontiguousarray(
        np.concatenate([rbf_ic, seqsep_ic, full_bias], axis=1).T)  # (128, N)

    hT_full = np.ascontiguousarray(h.T)                 # (D, N)

    gamma_a32 = gamma_a.astype(f)
    beta_a32 = beta_a.astype(f)
    bf = ml_dtypes.bfloat16
    shared = {
        "Wkp": Wkp, "Wb": Wb.astype(bf),
        "W1t": np.ascontiguousarray(W1[:D]),
        "Wgt": np.ascontiguousarray(Wg[:D]),
        "W2h": np.ascontiguousarray(RES * W2),
        "Wv": Wv.astype(bf), "Wo": Wo.astype(bf),
        "W1b": np.ascontiguousarray(W1[D:]).astype(bf),
        "Wgb": np.ascontiguousarray(Wg[D:]).astype(bf),
        "idn": np.eye(128, dtype=f),
        "ones128": np.ones((128, 1), f),
        "ncg": ncg, "cbv": cbv,
        "gb16": np.ascontiguousarray(np.broadcast_to(gamma_a32, (H, D))),
        "bb16": np.ascontiguousarray(np.broadcast_to(beta_a32, (H, D))),
        "hcv": np.ascontiguousarray(h[c].reshape(KT, 128).T),
        "b1v": np.ascontiguousarray(b1.reshape(KT, 128).T),
        "bgv": np.ascontiguousarray(bg.reshape(KT, 128).T),
        "b2v": np.ascontiguousarray((RES * b2).reshape(KT, 128).T),
        "epsv": np.full((1, 1), EPS, f),
    }
    in_maps = []
    for i in range(NCORES):
        r0 = i * NS
        m = dict(shared)
        m["hT"] = np.ascontiguousarray(hT_full[:, r0:r0 + NS])
        m["hN"] = h[r0:r0 + NS].astype(bf)
        m["bT"] = np.ascontiguousarray(bias_featT[:, r0:r0 + NS]).astype(bf)
        in_maps.append(m)

    nc = _get_nc()
    trace = bool(int(os.environ.get("KERNEL_TRACE", "0")))
    res = run_bass_kernel_spmd(nc, in_maps, core_ids=list(range(NCORES)),
                               trace=trace)
    LAST_RESULTS = res

    out = np.empty((N, D), f)
    for i in range(NCORES):
        out[i * NS:(i + 1) * NS] = res.results[i]["outT"].T
    hcn = res.results[0]["outC"].T.reshape(D)           # [m,p] -> flat
    out[c] = hcn
    return out



# revision 13
# speedup vs baseline: 1.1134x; 1.1134x over previous
"""Trainium2 Bass kernel for CenterGeoAttention (N=65536, D=1024, H=16).

Strategy (row-shard N across 8 cores, activations feature-major, all-bf16
matmul operands so FWL stays on):

  Pass 1a (per chunk): DMA h^T (bf16), square on DVE, LN stats via
    ones-matmuls, logits L = Wkp^T h accumulated; stats and L stored for
    the whole shard ([16,512] / [16,8192] chunk-on-partition layout).
  Batch stats: one sqrt/reciprocal for all 16 chunks (no per-chunk DVE
    reciprocal or Sqrt<->Exp activation-table churn).
  Pass 1b (per chunk): r/mr broadcast to 16 heads via tiny PE matmuls,
    softmax partials p, p*r, and G += (p*r)^T h (row-major h, bf16).
  AllReduce of [G | PRM | S] overlaps with pass 2's first two chunks of
    W1/Wg matmuls (their outputs staged to SBUF so the PE never waits on
    post-collective biases).
  Post: Gn -> ocv -> a0/g0 via host-folded (Wo@W1b), (Wo@Wgb): a0 =
    (W1b^T h_c + b1) + 0.5 (Wo W1b)^T ocv, removing h_c_new from the
    critical path (h_c_new itself is computed off-path for the center row).
  Pass 2 (per chunk): A = W1t^T h, Gt = Wgt^T h, silu/sigmoid with biases
    a0/g0, Cp = W2h^T silu, out = h + gate .* (Cp + b2/2).
"""

import os
import ml_dtypes
import numpy as np

import concourse.bass as bass
import concourse.bacc as bacc
import concourse.tile as tile
import concourse.mybir as mybir
from concourse.bass_utils import run_bass_kernel_spmd

F32 = mybir.dt.float32
F32R = mybir.dt.float32r
BF16 = mybir.dt.bfloat16
AF = mybir.ActivationFunctionType
OP = mybir.AluOpType
AX = mybir.AxisListType

NCORES = 8
N, D, H, HD, BIAS = 65536, 1024, 16, 64, 128
NS = N // NCORES            # 8192 rows per core
CH = 512                    # row-chunk
NCH = NS // CH              # 16 chunks
KT = D // 128               # 8 feature tiles
EPS = 1e-5
RES = 0.5

_CACHE = {}


def _selc():
    s = np.zeros((128, NCH * H), ml_dtypes.bfloat16)
    for c in range(NCH):
        s[:, c * H + c] = 1
    return s

LAST_RESULTS = None  # BassKernelResults from the most recent run (for test.py)


def _build(ncores=NCORES, variant="full"):
    nc = bacc.Bacc("TRN2", target_bir_lowering=False, debug=False,
                   num_devices=ncores)

    def din(name, shape, dt=BF16):
        return nc.dram_tensor(name, list(shape), dt, kind="ExternalInput").ap()

    # per-core tensors
    hT = din("hT", (D, NS))               # h_shard^T bf16
    hN = din("hN", (NS, D))               # h_shard natural bf16
    bT = din("bT", (BIAS, NS))            # bias_feat^T shard bf16
    # shared weights (all bf16)
    Wkp = din("Wkp", (D, H))
    Wb = din("Wb", (BIAS, H))
    W1t = din("W1t", (D, D))
    Wgt = din("Wgt", (D, D))
    W2h = din("W2h", (D, D))              # 0.5*W2
    Wv = din("Wv", (D, D))
    Wo = din("Wo", (D, D))
    WA = din("WA", (D, D))                # Wo @ W1[D:]
    WG = din("WG", (D, D))                # Wo @ Wg[D:]
    # small constants
    idn = din("idn", (128, 128), F32)
    ones128 = din("ones128", (128, 1))    # bf16
    ones16 = din("ones16", (1, H))      # bf16
    ncg = din("ncg", (H, 1), F32)         # -cg per head
    cbv = din("cbv", (H, 1), F32)         # cb per head (exp bias)
    gb16 = din("gb16", (H, D), F32)       # gamma_a broadcast rows
    bb16 = din("bb16", (H, D), F32)       # beta_a broadcast rows
    hcv = din("hcv", (128, KT), F32)      # h[c] as [p, m]
    a0c = din("a0c", (128, KT), F32)      # W1b^T h[c] + b1
    g0c = din("g0c", (128, KT), F32)      # Wgb^T h[c] + bg
    b2v = din("b2v", (128, KT), F32)      # 0.5*b2
    epsv = din("epsv", (NCH, 1), F32)     # eps per partition
    selc = din("selc", (128, NCH * H))    # one-hot col c in block c (bf16)

    outT = nc.dram_tensor("outT", [D, NS], F32, kind="ExternalOutput").ap()
    outC = nc.dram_tensor("outC", [128, KT], F32, kind="ExternalOutput").ap()

    with tile.TileContext(nc) as tc:
        with (
            tc.tile_pool(name="persist", bufs=1) as pp,
            tc.tile_pool(name="dram", bufs=1, space="DRAM") as dram,
        ):
            # ---- long-lived small tiles ----
            idn_s = pp.tile([128, 128], F32, tag="idn")
            nc.sync.dma_start(out=idn_s[:], in_=idn[:])
            ones_s = pp.tile([128, 1], BF16, tag="ones128")
            nc.sync.dma_start(out=ones_s[:], in_=ones128[:])
            ones16_s = pp.tile([1, H], BF16, tag="ones16")
            nc.sync.dma_start(out=ones16_s[:], in_=ones16[:])
            ncg_s = pp.tile([H, 1], F32, tag="ncg")
            nc.sync.dma_start(out=ncg_s[:], in_=ncg[:])
            cbv_s = pp.tile([H, 1], F32, tag="cbv")
            nc.sync.dma_start(out=cbv_s[:], in_=cbv[:])
            hcv_s = pp.tile([128, KT], F32, tag="hcv")
            nc.sync.dma_start(out=hcv_s[:], in_=hcv[:])
            a0c_s = pp.tile([128, KT], F32, tag="a0c")
            nc.sync.dma_start(out=a0c_s[:], in_=a0c[:])
            g0c_s = pp.tile([128, KT], F32, tag="g0c")
            nc.sync.dma_start(out=g0c_s[:], in_=g0c[:])
            b2v_s = pp.tile([128, KT], F32, tag="b2v")
            nc.sync.dma_start(out=b2v_s[:], in_=b2v[:])
            epsv_s = pp.tile([NCH, 1], F32, tag="epsv")
            nc.sync.dma_start(out=epsv_s[:], in_=epsv[:])
            Wkp_s = pp.tile([128, KT * H], BF16, tag="Wkp")
            for k in range(KT):
                nc.sync.dma_start(out=Wkp_s[:, k * H:(k + 1) * H],
                                  in_=Wkp[k * 128:(k + 1) * 128, :])
            Wb_s = pp.tile([BIAS, H], BF16, tag="Wb")
            nc.sync.dma_start(out=Wb_s[:], in_=Wb[:])
            selc_s = pp.tile([128, NCH * H], BF16, tag="selc")
            nc.sync.dma_start(out=selc_s[:], in_=selc[:])

            sCols = pp.tile([H, NCH], F32, tag="sCols")
            prmCols = pp.tile([H, NCH], F32, tag="prmCols")
            a0_s = pp.tile([128, KT], F32, tag="a0")
            g0_s = pp.tile([128, KT], F32, tag="g0")

            # resident pass-2 stationary weights (filled during pass 1)
            wres_cm = tc.tile_pool(name="p2w", bufs=1)
            wres = wres_cm.__enter__()
            W1t_s = wres.tile([128, KT * D], BF16, tag="W1t")
            Wgt_s = wres.tile([128, KT * D], BF16, tag="Wgt")
            W2h_s = wres.tile([128, KT * D], BF16, tag="W2h")

            # ======================= PASS 1a: stats + L =====================
            p1s_cm = tc.tile_pool(name="p1state", bufs=1)
            p1s = p1s_cm.__enter__()
            r16 = p1s.tile([NCH, CH], F32, tag="r16")
            r16b = p1s.tile([NCH, CH], BF16, tag="r16b")
            mr16b = p1s.tile([NCH, CH], BF16, tag="mr16b")
            r1 = p1s.tile([1, NS], BF16, tag="r1")
            mr1 = p1s.tile([1, NS], BF16, tag="mr1")
            Lall = p1s.tile([H, NS], F32, tag="Lall")

            psS_cm = tc.tile_pool(name="p1statps", bufs=1, space="PSUM")
            psS = psS_cm.__enter__()
            statm_ps = psS.tile([NCH, CH], F32, tag="statm")
            statq_ps = psS.tile([NCH, CH], F32, tag="statq")
            with (
                tc.tile_pool(name="p1a_sb", bufs=2) as sbA,
                tc.tile_pool(name="p1a_ps", bufs=2, space="PSUM") as psA1,
            ):
                for c in range(NCH):
                    c0 = c * CH
                    hTc = sbA.tile([128, KT * CH], BF16, tag="hTc")
                    for k in range(KT):
                        nc.sync.dma_start(
                            out=hTc[:, k * CH:(k + 1) * CH],
                            in_=hT[k * 128:(k + 1) * 128, c0:c0 + CH])
                    # spread resident-weight prefetch across chunks
                    if c < KT:
                        nc.sync.dma_start(
                            out=W1t_s[:, c * D:(c + 1) * D],
                            in_=W1t[c * 128:(c + 1) * 128, :])
                    else:
                        k2 = c - KT
                        nc.sync.dma_start(
                            out=Wgt_s[:, k2 * D:(k2 + 1) * D],
                            in_=Wgt[k2 * 128:(k2 + 1) * 128, :])
                    sq = sbA.tile([128, KT * CH], BF16, tag="sq")
                    nc.vector.tensor_mul(sq[:], hTc[:], hTc[:])
                    sel = selc_s[:, c * H:(c + 1) * H]
                    for k in range(KT):
                        nc.tensor.matmul(statm_ps[:], sel,
                                         hTc[:, k * CH:(k + 1) * CH],
                                         start=(c == 0 and k == 0),
                                         stop=(c == NCH - 1 and k == KT - 1))
                    for k in range(KT):
                        nc.tensor.matmul(statq_ps[:], sel,
                                         sq[:, k * CH:(k + 1) * CH],
                                         start=(c == 0 and k == 0),
                                         stop=(c == NCH - 1 and k == KT - 1))
                    L = psA1.tile([H, CH], F32, tag="L")
                    for k in range(KT):
                        nc.tensor.matmul(L[:], Wkp_s[:, k * H:(k + 1) * H],
                                         hTc[:, k * CH:(k + 1) * CH],
                                         start=(k == 0), stop=(k == KT - 1))
                    nc.scalar.copy(Lall[:, c0:c0 + CH], L[:])

            # ---- batched LN stats: r = 1/sqrt(var+eps), mr = mean*r ----
            statm_s = p1s.tile([NCH, CH], F32, tag="statm_s")
            nc.vector.tensor_copy(statm_s[:], statm_ps[:])
            msq = p1s.tile([NCH, CH], F32, tag="msq")
            nc.vector.tensor_mul(msq[:], statm_s[:], statm_ps[:])
            varD2 = p1s.tile([NCH, CH], F32, tag="varD2")
            nc.vector.scalar_tensor_tensor(
                varD2[:], statq_ps[:], float(D), msq[:],
                op0=OP.mult, op1=OP.subtract)
            sd16 = p1s.tile([NCH, CH], F32, tag="sd16")
            nc.scalar.activation(sd16[:], varD2[:], AF.Sqrt,
                                 bias=epsv_s[:, 0:1], scale=1.0 / (D * D))
            nc.vector.reciprocal_approx_fast(out=r16[:], in_=sd16[:])
            nc.vector.tensor_copy(r16b[:], r16[:])
            nc.vector.scalar_tensor_tensor(
                mr16b[:], statm_s[:], 1.0 / D, r16[:],
                op0=OP.mult, op1=OP.mult)
            psS_cm.__exit__(None, None, None)
            # rearrange [16, CH] chunk-on-partition -> [1, NS] partition 0
            nc.sync.dma_start(out=r1[0:1, 0:NS], in_=r16b[:, :])
            nc.sync.dma_start(out=mr1[0:1, 0:NS], in_=mr16b[:, :])

            # ======================= PASS 1b: softmax partials ==============
            psG_cm = tc.tile_pool(name="p1psG", bufs=1, space="PSUM")
            psG = psG_cm.__enter__()
            G = psG.tile([H, D], F32, tag="G")
            Gacc = p1s.tile([H, D], F32, tag="Gacc")
            with (
                tc.tile_pool(name="p1b_sb", bufs=2) as sbB,
                tc.tile_pool(name="p1b_ps", bufs=1, space="PSUM") as psB,
            ):
                for c in range(NCH):
                    c0 = c * CH
                    hNc = sbB.tile([128, 4 * D], BF16, tag="hNc")
                    for j in range(4):
                        nc.sync.dma_start(
                            out=hNc[:, j * D:(j + 1) * D],
                            in_=hN[c0 + j * 128:c0 + (j + 1) * 128, :])
                    bTc = sbB.tile([BIAS, CH], BF16, tag="bTc")
                    nc.sync.dma_start(out=bTc[:], in_=bT[:, c0:c0 + CH])
                    if c < KT:
                        nc.sync.dma_start(
                            out=W2h_s[:, c * D:(c + 1) * D],
                            in_=W2h[c * 128:(c + 1) * 128, :])

                    rb = psB.tile([H, CH], F32, tag="rb")
                    nc.tensor.matmul(rb[:], ones16_s[:],
                                     r1[0:1, c0:c0 + CH],
                                     start=True, stop=True)
                    mrb = psB.tile([H, CH], F32, tag="mrb")
                    nc.tensor.matmul(mrb[:], ones16_s[:],
                                     mr1[0:1, c0:c0 + CH],
                                     start=True, stop=True)
                    L2 = psB.tile([H, CH], F32, tag="L2")
                    nc.tensor.matmul(L2[:], Wb_s[:], bTc[:],
                                     start=True, stop=True)
                    t4 = sbB.tile([H, CH], F32, tag="t4")
                    nc.vector.scalar_tensor_tensor(
                        t4[:], mrb[:], ncg_s[:, 0:1], Lall[:, c0:c0 + CH],
                        op0=OP.mult, op1=OP.add)
                    t5a = sbB.tile([H, CH], F32, tag="t5a")
                    nc.vector.tensor_mul(t5a[:], t4[:], rb[:])
                    t5 = sbB.tile([H, CH], F32, tag="t5")
                    nc.vector.tensor_add(t5[:], t5a[:], L2[:])
                    pT = sbB.tile([H, CH], BF16, tag="pT")
                    nc.scalar.activation(pT[:], t5[:], AF.Exp,
                                         bias=cbv_s[:, 0:1],
                                         accum_out=sCols[:, c:c + 1])
                    prT = sbB.tile([H, CH], F32, tag="prT")
                    nc.vector.tensor_mul(prT[:], pT[:], rb[:])
                    prm_scr = sbB.tile([H, CH], F32, tag="prm_scr")
                    nc.vector.tensor_mul(prm_scr[:], pT[:], mrb[:])
                    nc.vector.reduce_sum(prmCols[:, c:c + 1], prm_scr[:],
                                         axis=AX.X)
                    tp = psB.tile([128, 4 * H], F32, tag="tp")
                    for j in range(4):
                        nc.tensor.transpose(
                            tp[:, j * H:(j + 1) * H],
                            prT[:, j * 128:(j + 1) * 128],
                            idn_s[0:16, 0:16])
                    pr_nat = sbB.tile([128, 4 * H], BF16, tag="pr_nat")
                    nc.vector.tensor_copy(pr_nat[:], tp[:])
                    for half in range(2):
                        for j in range(4):
                            nc.tensor.matmul(
                                G[:, half * CH:(half + 1) * CH],
                                pr_nat[:, j * H:(j + 1) * H],
                                hNc[:, j * D + half * CH:j * D + (half + 1) * CH],
                                start=(c == 0 and j == 0 and half == 0),
                                stop=(c == NCH - 1 and j == 3 and half == 1))
                nc.vector.tensor_copy(Gacc[:], G[:])

            psG_cm.__exit__(None, None, None)
            # ---- local partials -> AllReduce ----
            S16 = pp.tile([H, 1], F32, tag="S16")
            nc.vector.reduce_sum(S16[:], sCols[:], axis=AX.X)
            PRM16 = pp.tile([H, 1], F32, tag="PRM16")
            nc.vector.reduce_sum(PRM16[:], prmCols[:], axis=AX.X)

            arin = dram.tile([H, D + 2], F32, tag="arin")
            arout = dram.tile([H, D + 2], F32, tag="arout")
            nc.sync.dma_start(out=arin[:, 0:D], in_=Gacc[:])
            nc.sync.dma_start(out=arin[:, D:D + 1], in_=PRM16[:])
            nc.sync.dma_start(out=arin[:, D + 1:D + 2], in_=S16[:])
            if variant == "nocc":
                nc.sync.dma_start(out=arout[:], in_=arin[:])
            else:
                nc.gpsimd.collective_compute(
                    "AllReduce", OP.add,
                    replica_groups=[list(range(ncores))],
                    ins=[arin.opt()], outs=[arout.opt()])
            p1s_cm.__exit__(None, None, None)

            # ======================= PASS 2 =================================
            with (
                tc.tile_pool(name="p2sb", bufs=3) as sb3,
                tc.tile_pool(name="p2bs", bufs=2) as sbBs,
                tc.tile_pool(name="p2st", bufs=3) as sb4,
                tc.tile_pool(name="p2ps", bufs=2, space="PSUM") as ps3,
                tc.tile_pool(name="p2cps", bufs=2, space="PSUM") as psC,
                tc.tile_pool(name="wstream", bufs=2) as ws,
            ):
                def load_htc2(c):
                    c0 = c * CH
                    t = sb3.tile([128, KT * CH], BF16, tag="hTc2")
                    for k in range(KT):
                        nc.sync.dma_start(
                            out=t[:, k * CH:(k + 1) * CH],
                            in_=hT[k * 128:(k + 1) * 128, c0:c0 + CH])
                    return t

                def a_group(hTc2, consume):
                    """A = W1t^T h; consume(m, psum) drains each m-tile
                    immediately (copy for staging, silu for direct)."""
                    for m in range(KT):
                        A = ps3.tile([128, CH], F32, tag="A")
                        for k in range(KT):
                            nc.tensor.matmul(
                                A[:], W1t_s[:, k * D + m * 128:k * D + (m + 1) * 128],
                                hTc2[:, k * CH:(k + 1) * CH],
                                start=(k == 0), stop=(k == KT - 1))
                        consume(m, A[:])

                def gt_group(hTc2, consume):
                    for m in range(KT):
                        Gt = ps3.tile([128, CH], F32, tag="Gt")
                        for k in range(KT):
                            nc.tensor.matmul(
                                Gt[:], Wgt_s[:, k * D + m * 128:k * D + (m + 1) * 128],
                                hTc2[:, k * CH:(k + 1) * CH],
                                start=(k == 0), stop=(k == KT - 1))
                        consume(m, Gt[:])

                def cp_and_out(c, hTc2, Bs, gs):
                    c0 = c * CH
                    for m in range(KT):
                        Cp = psC.tile([128, CH], F32, tag="Cp")
                        for k in range(KT):
                            nc.tensor.matmul(
                                Cp[:], W2h_s[:, k * D + m * 128:k * D + (m + 1) * 128],
                                Bs[:, k * CH:(k + 1) * CH],
                                start=(k == 0), stop=(k == KT - 1))
                        t6 = sb4.tile([128, CH], F32, tag="t6")
                        nc.vector.scalar_tensor_tensor(
                            t6[:], Cp[:], b2v_s[:, m:m + 1],
                            gs[:, m * CH:(m + 1) * CH],
                            op0=OP.add, op1=OP.mult)
                        ot = sb4.tile([128, CH], F32, tag="ot")
                        nc.vector.tensor_add(
                            ot[:], t6[:], hTc2[:, m * CH:(m + 1) * CH])
                        nc.sync.dma_start(
                            out=outT[m * 128:(m + 1) * 128, c0:c0 + CH],
                            in_=ot[:])

                def stage_copy_scalar(dst):
                    return lambda m, ps: nc.scalar.copy(
                        dst[:, m * CH:(m + 1) * CH], ps)

                def stage_copy_vector(dst):
                    return lambda m, ps: nc.vector.tensor_copy(
                        dst[:, m * CH:(m + 1) * CH], ps)

                def silu_into(Bs):
                    return lambda m, src: nc.scalar.activation(
                        Bs[:, m * CH:(m + 1) * CH], src,
                        AF.Silu, bias=a0_s[:, m:m + 1])

                def sigm_into(gs):
                    return lambda m, src: nc.scalar.activation(
                        gs[:, m * CH:(m + 1) * CH], src,
                        AF.Sigmoid, bias=g0_s[:, m:m + 1])

                # -- stage chunks 0,1 (fills the AllReduce window) --
                with tc.tile_pool(name="p2stage", bufs=1) as stg:
                    ht0 = load_htc2(0)
                    bsb0 = stg.tile([128, KT * CH], BF16, tag="bsb0")
                    a_group(ht0, stage_copy_scalar(bsb0))
                    gsb0 = stg.tile([128, KT * CH], BF16, tag="gsb0")
                    gt_group(ht0, stage_copy_vector(gsb0))
                    ht1 = load_htc2(1)
                    bsb1 = stg.tile([128, KT * CH], BF16, tag="bsb1")
                    a_group(ht1, stage_copy_scalar(bsb1))
                    gsb1 = stg.tile([128, KT * CH], BF16, tag="gsb1")
                    gt_group(ht1, stage_copy_vector(gsb1))

                    # -- post-collective block --
                    with (
                        tc.tile_pool(name="postsb", bufs=1) as psb,
                        tc.tile_pool(name="postps", bufs=1, space="PSUM") as ps2,
                    ):
                        gb16_s = psb.tile([H, D], F32, tag="gb16")
                        nc.sync.dma_start(out=gb16_s[:], in_=gb16[:])
                        bb16_s = psb.tile([H, D], F32, tag="bb16")
                        nc.sync.dma_start(out=bb16_s[:], in_=bb16[:])
                        Wv_s = ws.tile([128, KT * D], BF16, tag="wstream")
                        for k in range(KT):
                            nc.sync.dma_start(out=Wv_s[:, k * D:(k + 1) * D],
                                              in_=Wv[k * 128:(k + 1) * 128, :])
                        WA_s = ws.tile([128, KT * D], BF16, tag="wstream")
                        for k in range(KT):
                            nc.sync.dma_start(out=WA_s[:, k * D:(k + 1) * D],
                                              in_=WA[k * 128:(k + 1) * 128, :])
                        Gar = psb.tile([H, D], F32, tag="Gar")
                        nc.sync.dma_start(out=Gar[:], in_=arout[:, 0:D])
                        PSar = psb.tile([H, 2], F32, tag="PSar")
                        nc.sync.dma_start(out=PSar[:], in_=arout[:, D:D + 2])
                        sr = psb.tile([H, 1], F32, tag="sr")
                        nc.vector.reciprocal_approx_fast(
                            out=sr[:], in_=PSar[:, 1:2])
                        Gn = psb.tile([H, D], F32, tag="Gn")
                        nc.vector.tensor_scalar_sub(Gn[:], Gar[:],
                                                    PSar[:, 0:1])
                        nc.vector.tensor_mul(Gn[:], Gn[:], gb16_s[:])
                        nc.vector.scalar_tensor_tensor(
                            Gn[:], Gn[:], sr[:, 0:1], bb16_s[:],
                            op0=OP.mult, op1=OP.add)
                        tpg = ps2.tile([128, KT * H], F32, tag="post1")
                        for m in range(KT):
                            nc.tensor.transpose(
                                tpg[:, m * H:(m + 1) * H],
                                Gn[:, m * 128:(m + 1) * 128],
                                idn_s[0:16, 0:16])
                        GnT = psb.tile([128, KT * H], BF16, tag="GnT")
                        nc.vector.tensor_copy(GnT[:], tpg[:])

                        OCp = ps2.tile([128, KT * H], F32, tag="post1")
                        for m in range(KT):
                            for k in range(KT):
                                nc.tensor.matmul(
                                    OCp[:, m * H:(m + 1) * H],
                                    Wv_s[:, k * D + m * 128:k * D + (m + 1) * 128],
                                    GnT[:, k * H:(k + 1) * H],
                                    start=(k == 0), stop=(k == KT - 1))
                        ocv = psb.tile([128, KT], BF16, tag="ocv")
                        for m in range(KT):
                            if m % 2 == 0:
                                nc.vector.tensor_copy(
                                    ocv[0:64, m:m + 1],
                                    OCp[0:64, m * H + 2 * m:m * H + 2 * m + 1])
                                nc.vector.tensor_copy(
                                    ocv[64:128, m:m + 1],
                                    OCp[64:128, m * H + 2 * m + 1:m * H + 2 * m + 2])
                            else:
                                nc.scalar.copy(
                                    ocv[0:64, m:m + 1],
                                    OCp[0:64, m * H + 2 * m:m * H + 2 * m + 1])
                                nc.scalar.copy(
                                    ocv[64:128, m:m + 1],
                                    OCp[64:128, m * H + 2 * m + 1:m * H + 2 * m + 2])

                        a0p = ps2.tile([128, KT], F32, tag="smv")
                        for m in range(KT):
                            for k in range(KT):
                                nc.tensor.matmul(
                                    a0p[:, m:m + 1],
                                    WA_s[:, k * D + m * 128:k * D + (m + 1) * 128],
                                    ocv[:, k:k + 1],
                                    start=(k == 0), stop=(k == KT - 1))
                        nc.vector.scalar_tensor_tensor(
                            a0_s[:], a0p[:], RES, a0c_s[:],
                            op0=OP.mult, op1=OP.add)

                        WG_s = ws.tile([128, KT * D], BF16, tag="wstream")
                        for k in range(KT):
                            nc.sync.dma_start(out=WG_s[:, k * D:(k + 1) * D],
                                              in_=WG[k * 128:(k + 1) * 128, :])
                        g0p = ps2.tile([128, KT], F32, tag="smv")
                        for m in range(KT):
                            for k in range(KT):
                                nc.tensor.matmul(
                                    g0p[:, m:m + 1],
                                    WG_s[:, k * D + m * 128:k * D + (m + 1) * 128],
                                    ocv[:, k:k + 1],
                                    start=(k == 0), stop=(k == KT - 1))
                        nc.vector.scalar_tensor_tensor(
                            g0_s[:], g0p[:], RES, g0c_s[:],
                            op0=OP.mult, op1=OP.add)

                        # h_c_new for the center row (off critical path)
                        Wo_s = ws.tile([128, KT * D], BF16, tag="wstream")
                        for k in range(KT):
                            nc.sync.dma_start(out=Wo_s[:, k * D:(k + 1) * D],
                                              in_=Wo[k * 128:(k + 1) * 128, :])
                        hcp = ps2.tile([128, KT], F32, tag="smv")
                        for m in range(KT):
                            for k in range(KT):
                                nc.tensor.matmul(
                                    hcp[:, m:m + 1],
                                    Wo_s[:, k * D + m * 128:k * D + (m + 1) * 128],
                                    ocv[:, k:k + 1],
                                    start=(k == 0), stop=(k == KT - 1))
                        hcn_sb = psb.tile([128, KT], F32, tag="hcn")
                        nc.vector.scalar_tensor_tensor(
                            hcn_sb[:], hcp[:], RES, hcv_s[:],
                            op0=OP.mult, op1=OP.add)
                        nc.sync.dma_start(out=outC[:], in_=hcn_sb[:])

                    # -- consume staged chunks --
                    for c, (ht, bsb, gsb) in ((0, (ht0, bsb0, gsb0)),
                                              (1, (ht1, bsb1, gsb1))):
                        Bs = sbBs.tile([128, KT * CH], BF16, tag="Bs")
                        fb = silu_into(Bs)
                        for m in range(KT):
                            fb(m, bsb[:, m * CH:(m + 1) * CH])
                        gs = sbBs.tile([128, KT * CH], BF16, tag="gs")
                        fg = sigm_into(gs)
                        for m in range(KT):
                            fg(m, gsb[:, m * CH:(m + 1) * CH])
                        cp_and_out(c, ht, Bs, gs)

                # -- remaining chunks: direct PSUM path --
                for c in range(2, NCH):
                    htc = load_htc2(c)
                    Bs = sbBs.tile([128, KT * CH], BF16, tag="Bs")
                    a_group(htc, silu_into(Bs))
                    gs = sbBs.tile([128, KT * CH], BF16, tag="gs")
                    gt_group(htc, sigm_into(gs))
                    cp_and_out(c, htc, Bs, gs)

            wres_cm.__exit__(None, None, None)
    nc.compile()
    return nc


def _get_nc():
    if "nc" not in _CACHE:
        _CACHE["nc"] = _build()
    return _CACHE["nc"]


def kernel(h, center_idx, rbf_ic, seqsep_ic, nbr_idx, local_bias,
           gamma_c, beta_c, gamma_a, beta_a,
           Wq, Wk, Wv, Wo, Wb, W1, b1, W2, b2, Wg, bg):
    global LAST_RESULTS
    f = np.float32
    bf = ml_dtypes.bfloat16
    h = np.asarray(h, f)
    c = int(center_idx)
    rbf_ic = np.asarray(rbf_ic, f)
    seqsep_ic = np.asarray(seqsep_ic, f)
    nbr_idx = np.asarray(nbr_idx)
    local_bias = np.asarray(local_bias, f)
    gamma_c = np.asarray(gamma_c, np.float64)
    beta_c = np.asarray(beta_c, np.float64)
    gamma_a = np.asarray(gamma_a, np.float64)
    beta_a = np.asarray(beta_a, np.float64)
    Wq = np.asarray(Wq, f); Wk = np.asarray(Wk, f); Wv = np.asarray(Wv, f)
    Wo = np.asarray(Wo, f); Wb = np.asarray(Wb, f)
    W1 = np.asarray(W1, f); b1 = np.asarray(b1, f)
    W2 = np.asarray(W2, f); b2 = np.asarray(b2, f)
    Wg = np.asarray(Wg, f); bg = np.asarray(bg, f)

    # ---- host algebra (tiny, no big matmuls beyond two DxD folds) ----
    hc = h[c].astype(np.float64)
    hcl = (hc - hc.mean()) / np.sqrt(hc.var() + EPS) * gamma_c + beta_c
    q = (hcl @ Wq.astype(np.float64)).reshape(H, HD)
    Qm = np.zeros((D, H), np.float64)
    for hh in range(H):
        Qm[hh * HD:(hh + 1) * HD, hh] = q[hh] / np.sqrt(HD)
    Wk1 = Wk.astype(np.float64) @ Qm                    # (D, 16)
    Wkp = (Wk1 * gamma_a[:, None]).astype(bf)
    ncg = (-(Wk1 * gamma_a[:, None]).sum(0)).astype(f).reshape(H, 1)
    cbv = (Wk1 * beta_a[:, None]).sum(0).astype(f).reshape(H, 1)

    full_bias = np.zeros((N, local_bias.shape[1]), f)
    full_bias[nbr_idx] = local_bias
    bias_featT = np.ascontiguousarray(
        np.concatenate([rbf_ic, seqsep_ic, full_bias], axis=1).T)  # (128, N)

    hT_full = np.ascontiguousarray(h.T).astype(bf)      # (D, N) bf16

    W1b = W1[D:]
    Wgb = Wg[D:]
    WA = (Wo.astype(np.float64) @ W1b.astype(np.float64)).astype(bf)
    WG = (Wo.astype(np.float64) @ Wgb.astype(np.float64)).astype(bf)
    a0c = (h[c].astype(np.float64) @ W1b.astype(np.float64) + b1).astype(f)
    g0c = (h[c].astype(np.float64) @ Wgb.astype(np.float64) + bg).astype(f)

    gamma_a32 = gamma_a.astype(f)
    beta_a32 = beta_a.astype(f)
    shared = {
        "Wkp": Wkp, "Wb": Wb.astype(bf),
        "W1t": np.ascontiguousarray(W1[:D]).astype(bf),
        "Wgt": np.ascontiguousarray(Wg[:D]).astype(bf),
        "W2h": np.ascontiguousarray(RES * W2).astype(bf),
        "Wv": Wv.astype(bf), "Wo": Wo.astype(bf),
        "WA": WA, "WG": WG,
        "idn": np.eye(128, dtype=f),
        "ones128": np.ones((128, 1), bf),
        "ones16": np.ones((1, H), bf),
        "ncg": ncg, "cbv": cbv,
        "gb16": np.ascontiguousarray(np.broadcast_to(gamma_a32, (H, D))),
        "bb16": np.ascontiguousarray(np.broadcast_to(beta_a32, (H, D))),
        "hcv": np.ascontiguousarray(h[c].reshape(KT, 128).T),
        "a0c": np.ascontiguousarray(a0c.reshape(KT, 128).T),
        "g0c": np.ascontiguousarray(g0c.reshape(KT, 128).T),
        "b2v": np.ascontiguousarray((RES * b2).reshape(KT, 128).T),
        "epsv": np.full((NCH, 1), EPS, f),
        "selc": _selc(),
    }
    in_maps = []
    for i in range(NCORES):
        r0 = i * NS
        m = dict(shared)
        m["hT"] = np.ascontiguousarray(hT_full[:, r0:r0 + NS])
        m["hN"] = h[r0:r0 + NS].astype(bf)
        m["bT"] = np.ascontiguousarray(bias_featT[:, r0:r0 + NS]).astype(bf)
        in_maps.append(m)

    nc = _get_nc()
    trace = bool(int(os.environ.get("KERNEL_TRACE", "0")))
    res = run_bass_kernel_spmd(nc, in_maps, core_ids=list(range(NCORES)),
                               trace=trace)
    LAST_RESULTS = res

    out = np.empty((N, D), f)
    for i in range(NCORES):
        out[i * NS:(i + 1) * NS] = res.results[i]["outT"].T
    hcn = res.results[0]["outC"].T.reshape(D)           # [m,p] -> flat
    out[c] = hcn
    return out


# revision 16
# speedup vs baseline: 1.7452x; 1.5674x over previous
"""Trainium2 Bass kernel for CenterGeoAttention (N=65536, D=1024, H=16).

Strategy (row-shard N across 8 cores, activations feature-major, all-bf16
matmul operands so FWL stays on):

  Pass 1a (per chunk): DMA h^T (bf16), square on DVE, LN stats via
    ones-matmuls, logits L = Wkp^T h accumulated; stats and L stored for
    the whole shard ([16,512] / [16,8192] chunk-on-partition layout).
  Batch stats: one sqrt/reciprocal for all 16 chunks (no per-chunk DVE
    reciprocal or Sqrt<->Exp activation-table churn).
  Pass 1b (per chunk): r/mr broadcast to 16 heads via tiny PE matmuls,
    softmax partials p, p*r, and G += (p*r)^T h (row-major h, bf16).
  AllReduce of [G | PRM | S] overlaps with pass 2's first two chunks of
    W1/Wg matmuls (their outputs staged to SBUF so the PE never waits on
    post-collective biases).
  Post: Gn -> ocv -> a0/g0 via host-folded (Wo@W1b), (Wo@Wgb): a0 =
    (W1b^T h_c + b1) + 0.5 (Wo W1b)^T ocv, removing h_c_new from the
    critical path (h_c_new itself is computed off-path for the center row).
  Pass 2 (per chunk): A = W1t^T h, Gt = Wgt^T h, silu/sigmoid with biases
    a0/g0, Cp = W2h^T silu, out = h + gate .* (Cp + b2/2).
"""

import os
import ml_dtypes
import numpy as np

import concourse.bass as bass
import concourse.bacc as bacc
import concourse.tile as tile
import concourse.mybir as mybir
from concourse.bass_utils import run_bass_kernel_spmd

F32 = mybir.dt.float32
F32R = mybir.dt.float32r
BF16 = mybir.dt.bfloat16
FP8 = mybir.dt.float8e4
DR = mybir.MatmulPerfMode.DoubleRow
AF = mybir.ActivationFunctionType
OP = mybir.AluOpType
AX = mybir.AxisListType

NCORES = 8
N, D, H, HD, BIAS = 65536, 1024, 16, 64, 128
NS = N // NCORES            # 8192 rows per core
CH = 512                    # row-chunk
NCH = NS // CH              # 16 chunks
KT = D // 128               # 8 feature tiles
EPS = 1e-5
RES = 0.5

_CACHE = {}


def _selc():
    f8 = ml_dtypes.float8_e4m3fn
    s = np.zeros((128, NCH * 2 * H), f8)
    for c in range(NCH):
        s[:, c * 2 * H + c] = 1
        s[:, c * 2 * H + H + c] = 1
    return s

LAST_RESULTS = None  # BassKernelResults from the most recent run (for test.py)


def _build(ncores=NCORES, variant="full"):
    nc = bacc.Bacc("TRN2", target_bir_lowering=False, debug=False,
                   num_devices=ncores)

    def din(name, shape, dt=BF16):
        return nc.dram_tensor(name, list(shape), dt, kind="ExternalInput").ap()

    # per-core tensors
    hT8 = din("hT8", (D, NS), FP8)        # h_shard^T fp8 (matmul operand)
    hTb = din("hTb", (D, NS))             # h_shard^T bf16 (residual)
    hN8 = din("hN8", (NS, D), FP8)        # h_shard natural fp8
    bT = din("bT", (BIAS, NS))            # bias_feat^T shard bf16
    # shared weights
    Wkp8 = din("Wkp8", (D, H), FP8)       # 256 * Wkp
    Wb = din("Wb", (BIAS, 128))           # bf16, padded to 128 cols
    W1t8 = din("W1t8", (D, D), FP8)       # 16 * W1[:D]
    Wgt8 = din("Wgt8", (D, D), FP8)       # 16 * Wg[:D]
    W2h8 = din("W2h8", (D, D), FP8)       # 16 * 0.5 * W2
    Wv = din("Wv", (D, D))
    Wo = din("Wo", (D, D))
    WA = din("WA", (D, D))                # Wo @ W1[D:]
    WG = din("WG", (D, D))                # Wo @ Wg[D:]
    # small constants
    idn = din("idn", (128, 128), F32)
    ones16 = din("ones16", (1, 128))      # bf16, padded to 128 cols
    ncg = din("ncg", (H, 1), F32)         # -256*cg per head
    cbv = din("cbv", (H, 1), F32)         # cb per head (exp bias)
    gb16 = din("gb16", (H, D), F32)       # gamma_a broadcast rows
    bb16 = din("bb16", (H, D), F32)       # beta_a broadcast rows
    hcv = din("hcv", (128, KT), F32)      # h[c] as [p, m]
    a0c = din("a0c", (128, KT), F32)      # W1b^T h[c] + b1
    g0c = din("g0c", (128, KT), F32)      # Wgb^T h[c] + bg
    b2v = din("b2v", (128, KT), F32)      # 16*0.5*b2
    epsv = din("epsv", (NCH, 1), F32)     # eps per partition
    selc8 = din("selc8", (128, NCH * 2 * H), FP8)  # one-hot col c, doubled

    outT = nc.dram_tensor("outT", [D, NS], F32, kind="ExternalOutput").ap()
    outC = nc.dram_tensor("outC", [128, KT], F32, kind="ExternalOutput").ap()

    with tile.TileContext(nc) as tc:
        with (
            tc.tile_pool(name="persist", bufs=1) as pp,
            tc.tile_pool(name="dram", bufs=1, space="DRAM") as dram,
        ):
            # ---- long-lived small tiles ----
            idn_s = pp.tile([128, 128], F32, tag="idn")
            nc.sync.dma_start(out=idn_s[:], in_=idn[:])
            ones16_s = pp.tile([1, 128], BF16, tag="ones16")
            nc.sync.dma_start(out=ones16_s[:], in_=ones16[:])
            ncg_s = pp.tile([H, 1], F32, tag="ncg")
            nc.sync.dma_start(out=ncg_s[:], in_=ncg[:])
            cbv_s = pp.tile([H, 1], F32, tag="cbv")
            nc.sync.dma_start(out=cbv_s[:], in_=cbv[:])
            hcv_s = pp.tile([128, KT], F32, tag="hcv")
            nc.sync.dma_start(out=hcv_s[:], in_=hcv[:])
            a0c_s = pp.tile([128, KT], F32, tag="a0c")
            nc.sync.dma_start(out=a0c_s[:], in_=a0c[:])
            g0c_s = pp.tile([128, KT], F32, tag="g0c")
            nc.sync.dma_start(out=g0c_s[:], in_=g0c[:])
            b2v_s = pp.tile([128, KT], F32, tag="b2v")
            nc.sync.dma_start(out=b2v_s[:], in_=b2v[:])
            epsv_s = pp.tile([NCH, 1], F32, tag="epsv")
            nc.sync.dma_start(out=epsv_s[:], in_=epsv[:])
            Wkp_s = pp.tile([128, KT, H], FP8, tag="Wkp")
            for k in range(KT):
                nc.sync.dma_start(out=Wkp_s[:, k:k + 1, :],
                                  in_=Wkp8[k * 128:(k + 1) * 128, :])
            Wb_s = pp.tile([BIAS, 128], BF16, tag="Wb")
            nc.sync.dma_start(out=Wb_s[:], in_=Wb[:])
            selc_s = pp.tile([128, NCH, 2, H], FP8, tag="selc")
            nc.sync.dma_start(out=selc_s[:], in_=selc8[:])

            sCols = pp.tile([H, NCH], F32, tag="sCols")
            prmCols = pp.tile([H, NCH], F32, tag="prmCols")
            a0_s = pp.tile([128, KT], F32, tag="a0")
            g0_s = pp.tile([128, KT], F32, tag="g0")

            # resident pass-2 stationary weights (filled during pass 1)
            wres_cm = tc.tile_pool(name="p2w", bufs=1)
            wres = wres_cm.__enter__()
            W1t_s = wres.tile([128, KT, D], FP8, tag="W1t")
            Wgt_s = wres.tile([128, KT, D], FP8, tag="Wgt")
            W2h_s = wres.tile([128, KT, D], FP8, tag="W2h")

            # ======================= PASS 1a: stats + L =====================
            p1s_cm = tc.tile_pool(name="p1state", bufs=1)
            p1s = p1s_cm.__enter__()
            r16 = p1s.tile([NCH, CH], F32, tag="r16")
            r16b = p1s.tile([NCH, CH], BF16, tag="r16b")
            mr16b = p1s.tile([NCH, CH], BF16, tag="mr16b")
            r1 = p1s.tile([1, NS], BF16, tag="r1")
            mr1 = p1s.tile([1, NS], BF16, tag="mr1")
            Lall = p1s.tile([H, NS], F32, tag="Lall")

            psS_cm = tc.tile_pool(name="p1statps", bufs=1, space="PSUM")
            psS = psS_cm.__enter__()
            statm_ps = psS.tile([NCH, CH], F32, tag="statm")
            statq_ps = psS.tile([NCH, CH], F32, tag="statq")
            with (
                tc.tile_pool(name="p1a_sb", bufs=2) as sbA,
                tc.tile_pool(name="p1a_ps", bufs=2, space="PSUM") as psA1,
            ):
                for c in range(NCH):
                    c0 = c * CH
                    hTc = sbA.tile([128, KT, CH], FP8, tag="hTc")
                    for k in range(KT):
                        nc.sync.dma_start(
                            out=hTc[:, k:k + 1, :],
                            in_=hT8[k * 128:(k + 1) * 128, c0:c0 + CH])
                    # spread resident-weight prefetch across chunks
                    if c < KT:
                        nc.sync.dma_start(
                            out=W1t_s[:, c:c + 1, :],
                            in_=W1t8[c * 128:(c + 1) * 128, :])
                    else:
                        k2 = c - KT
                        nc.sync.dma_start(
                            out=Wgt_s[:, k2:k2 + 1, :],
                            in_=Wgt8[k2 * 128:(k2 + 1) * 128, :])
                    sq = sbA.tile([128, KT, CH], FP8, tag="sq")
                    nc.vector.tensor_mul(sq[:], hTc[:], hTc[:])
                    sel = selc_s[:, c, :, :]
                    for k in range(0, KT, 2):
                        nc.tensor.matmul(statm_ps[:], sel,
                                         hTc[:, k:k + 2, :],
                                         start=(c == 0 and k == 0),
                                         stop=(c == NCH - 1 and k == KT - 2),
                                         perf_mode=DR)
                    for k in range(0, KT, 2):
                        nc.tensor.matmul(statq_ps[:], sel,
                                         sq[:, k:k + 2, :],
                                         start=(c == 0 and k == 0),
                                         stop=(c == NCH - 1 and k == KT - 2),
                                         perf_mode=DR)
                    L = psA1.tile([H, CH], F32, tag="L")
                    for k in range(0, KT, 2):
                        nc.tensor.matmul(L[:], Wkp_s[:, k:k + 2, :],
                                         hTc[:, k:k + 2, :],
                                         start=(k == 0), stop=(k == KT - 2),
                                         perf_mode=DR)
                    nc.scalar.copy(Lall[:, c0:c0 + CH], L[:])

            # ---- batched LN stats: r = 1/sqrt(var+eps), mr = mean*r ----
            statm_s = p1s.tile([NCH, CH], F32, tag="statm_s")
            nc.vector.tensor_copy(statm_s[:], statm_ps[:])
            msq = p1s.tile([NCH, CH], F32, tag="msq")
            nc.vector.tensor_mul(msq[:], statm_s[:], statm_ps[:])
            varD2 = p1s.tile([NCH, CH], F32, tag="varD2")
            nc.vector.scalar_tensor_tensor(
                varD2[:], statq_ps[:], float(D), msq[:],
                op0=OP.mult, op1=OP.subtract)
            sd16 = p1s.tile([NCH, CH], F32, tag="sd16")
            nc.scalar.activation(sd16[:], varD2[:], AF.Sqrt,
                                 bias=epsv_s[:, 0:1], scale=1.0 / (D * D))
            nc.vector.reciprocal_approx_fast(out=r16[:], in_=sd16[:])
            nc.vector.tensor_copy(r16b[:], r16[:])
            nc.vector.scalar_tensor_tensor(
                mr16b[:], statm_s[:], 1.0 / D, r16[:],
                op0=OP.mult, op1=OP.mult)
            psS_cm.__exit__(None, None, None)
            # rearrange [16, CH] chunk-on-partition -> [1, NS] partition 0
            nc.sync.dma_start(out=r1[0:1, 0:NS], in_=r16b[:, :])
            nc.sync.dma_start(out=mr1[0:1, 0:NS], in_=mr16b[:, :])

            # ======================= PASS 1b: softmax partials ==============
            psG_cm = tc.tile_pool(name="p1psG", bufs=1, space="PSUM")
            psG = psG_cm.__enter__()
            G = psG.tile([H, D], F32, tag="G")
            Gacc = p1s.tile([H, D], F32, tag="Gacc")
            with (
                tc.tile_pool(name="p1b_sb", bufs=2) as sbB,
                tc.tile_pool(name="p1b_ps", bufs=1, space="PSUM") as psB,
            ):
                for c in range(NCH):
                    c0 = c * CH
                    hNc = sbB.tile([128, 4, D], FP8, tag="hNc")
                    for j in range(4):
                        nc.sync.dma_start(
                            out=hNc[:, j:j + 1, :],
                            in_=hN8[c0 + j * 128:c0 + (j + 1) * 128, :])
                    bTc = sbB.tile([BIAS, CH], BF16, tag="bTc")
                    nc.sync.dma_start(out=bTc[:], in_=bT[:, c0:c0 + CH])
                    if c < KT:
                        nc.sync.dma_start(
                            out=W2h_s[:, c:c + 1, :],
                            in_=W2h8[c * 128:(c + 1) * 128, :])

                    rbf = psB.tile([128, CH], F32, tag="rb")
                    nc.tensor.matmul(rbf[:], ones16_s[:],
                                     r1[0:1, c0:c0 + CH],
                                     start=True, stop=True)
                    rb = rbf[0:H, :]
                    mrbf = psB.tile([128, CH], F32, tag="mrb")
                    nc.tensor.matmul(mrbf[:], ones16_s[:],
                                     mr1[0:1, c0:c0 + CH],
                                     start=True, stop=True)
                    mrb = mrbf[0:H, :]
                    L2f = psB.tile([128, CH], F32, tag="L2")
                    nc.tensor.matmul(L2f[:], Wb_s[:], bTc[:],
                                     start=True, stop=True)
                    L2 = L2f[0:H, :]
                    t4 = sbB.tile([H, CH], F32, tag="t4")
                    nc.vector.scalar_tensor_tensor(
                        t4[:], mrb, ncg_s[:, 0:1], Lall[:, c0:c0 + CH],
                        op0=OP.mult, op1=OP.add)
                    t5a = sbB.tile([H, CH], F32, tag="t5a")
                    nc.vector.tensor_mul(t5a[:], t4[:], rb)
                    t5 = sbB.tile([H, CH], F32, tag="t5")
                    nc.vector.scalar_tensor_tensor(
                        t5[:], t5a[:], 1.0 / 256.0, L2,
                        op0=OP.mult, op1=OP.add)
                    pT = sbB.tile([H, CH], BF16, tag="pT")
                    nc.scalar.activation(pT[:], t5[:], AF.Exp,
                                         bias=cbv_s[:, 0:1],
                                         accum_out=sCols[:, c:c + 1])
                    prT = sbB.tile([H, CH], F32, tag="prT")
                    nc.vector.tensor_mul(prT[:], pT[:], rb)
                    prm_scr = sbB.tile([H, CH], F32, tag="prm_scr")
                    nc.vector.tensor_mul(prm_scr[:], pT[:], mrb)
                    nc.vector.reduce_sum(prmCols[:, c:c + 1], prm_scr[:],
                                         axis=AX.X)
                    tp = psB.tile([128, 4 * H], F32, tag="tp")
                    for j in range(4):
                        nc.tensor.transpose(
                            tp[:, j * H:(j + 1) * H],
                            prT[:, j * 128:(j + 1) * 128],
                            idn_s[0:16, 0:16])
                    pr_nat = sbB.tile([128, 4, H], FP8, tag="pr_nat")
                    nc.vector.tensor_copy(pr_nat[:, :, :], tp[:])
                    for half in range(2):
                        for j in range(0, 4, 2):
                            nc.tensor.matmul(
                                G[:, half * CH:(half + 1) * CH],
                                pr_nat[:, j:j + 2, :],
                                hNc[:, j:j + 2, half * CH:(half + 1) * CH],
                                start=(c == 0 and j == 0 and half == 0),
                                stop=(c == NCH - 1 and j == 2 and half == 1),
                                perf_mode=DR)
                nc.vector.tensor_copy(Gacc[:], G[:])

            psG_cm.__exit__(None, None, None)
            # ---- local partials -> AllReduce ----
            S16 = pp.tile([H, 1], F32, tag="S16")
            nc.vector.reduce_sum(S16[:], sCols[:], axis=AX.X)
            PRM16 = pp.tile([H, 1], F32, tag="PRM16")
            nc.vector.reduce_sum(PRM16[:], prmCols[:], axis=AX.X)

            arin = dram.tile([H, D + 2], F32, tag="arin")
            arout = dram.tile([H, D + 2], F32, tag="arout")
            nc.sync.dma_start(out=arin[:, 0:D], in_=Gacc[:])
            nc.sync.dma_start(out=arin[:, D:D + 1], in_=PRM16[:])
            nc.sync.dma_start(out=arin[:, D + 1:D + 2], in_=S16[:])
            if variant == "nocc":
                nc.sync.dma_start(out=arout[:], in_=arin[:])
            else:
                nc.gpsimd.collective_compute(
                    "AllReduce", OP.add,
                    replica_groups=[list(range(ncores))],
                    ins=[arin.opt()], outs=[arout.opt()])
            p1s_cm.__exit__(None, None, None)

            # ======================= PASS 2 =================================
            with (
                tc.tile_pool(name="p2sb", bufs=3) as sb3,
                tc.tile_pool(name="p2bs", bufs=2) as sbBs,
                tc.tile_pool(name="p2st", bufs=3) as sb4,
                tc.tile_pool(name="p2ps", bufs=2, space="PSUM") as ps3,
                tc.tile_pool(name="p2cps", bufs=2, space="PSUM") as psC,
                tc.tile_pool(name="wstream", bufs=2) as ws,
            ):
                def load_htc2(c):
                    c0 = c * CH
                    t = sb3.tile([128, KT, CH], FP8, tag="hTc2")
                    for k in range(KT):
                        nc.sync.dma_start(
                            out=t[:, k:k + 1, :],
                            in_=hT8[k * 128:(k + 1) * 128, c0:c0 + CH])
                    tb = sb3.tile([128, KT * CH], BF16, tag="hTb2")
                    for k in range(KT):
                        nc.sync.dma_start(
                            out=tb[:, k * CH:(k + 1) * CH],
                            in_=hTb[k * 128:(k + 1) * 128, c0:c0 + CH])
                    return t, tb

                def a_group(hTc2, consume):
                    """A = W1t^T h; consume(m, psum) drains each m-tile
                    immediately (copy for staging, silu for direct)."""
                    for m in range(KT):
                        A = ps3.tile([128, CH], F32, tag="A")
                        for k in range(0, KT, 2):
                            nc.tensor.matmul(
                                A[:], W1t_s[:, k:k + 2, m * 128:(m + 1) * 128],
                                hTc2[:, k:k + 2, :],
                                start=(k == 0), stop=(k == KT - 2),
                                perf_mode=DR)
                        consume(m, A[:])

                def gt_group(hTc2, consume):
                    for m in range(KT):
                        Gt = ps3.tile([128, CH], F32, tag="Gt")
                        for k in range(0, KT, 2):
                            nc.tensor.matmul(
                                Gt[:], Wgt_s[:, k:k + 2, m * 128:(m + 1) * 128],
                                hTc2[:, k:k + 2, :],
                                start=(k == 0), stop=(k == KT - 2),
                                perf_mode=DR)
                        consume(m, Gt[:])

                def cp_and_out(c, hTb2, Bs, gs):
                    c0 = c * CH
                    for m in range(KT):
                        Cp = psC.tile([128, CH], F32, tag="Cp")
                        for k in range(0, KT, 2):
                            nc.tensor.matmul(
                                Cp[:], W2h_s[:, k:k + 2, m * 128:(m + 1) * 128],
                                Bs[:, k:k + 2, :],
                                start=(k == 0), stop=(k == KT - 2),
                                perf_mode=DR)
                        t6 = sb4.tile([128, CH], F32, tag="t6")
                        nc.vector.scalar_tensor_tensor(
                            t6[:], Cp[:], b2v_s[:, m:m + 1],
                            gs[:, m * CH:(m + 1) * CH],
                            op0=OP.add, op1=OP.mult)
                        ot = sb4.tile([128, CH], F32, tag="ot")
                        nc.vector.scalar_tensor_tensor(
                            ot[:], t6[:], 1.0 / 16.0,
                            hTb2[:, m * CH:(m + 1) * CH],
                            op0=OP.mult, op1=OP.add)
                        nc.sync.dma_start(
                            out=outT[m * 128:(m + 1) * 128, c0:c0 + CH],
                            in_=ot[:])

                def stage_copy_scalar(dst):
                    return lambda m, ps: nc.scalar.copy(
                        dst[:, m * CH:(m + 1) * CH], ps)

                def stage_copy_vector(dst):
                    return lambda m, ps: nc.vector.tensor_copy(
                        dst[:, m * CH:(m + 1) * CH], ps)

                def silu_into(Bs):
                    return lambda m, src: nc.scalar.activation(
                        Bs[:, m, :], src,
                        AF.Silu, bias=a0_s[:, m:m + 1], scale=1.0 / 16.0)

                def sigm_into(gs):
                    return lambda m, src: nc.scalar.activation(
                        gs[:, m * CH:(m + 1) * CH], src,
                        AF.Sigmoid, bias=g0_s[:, m:m + 1], scale=1.0 / 16.0)

                # -- stage chunks 0,1 (fills the AllReduce window) --
                with tc.tile_pool(name="p2stage", bufs=1) as stg:
                    ht0, htb0 = load_htc2(0)
                    bsb0 = stg.tile([128, KT * CH], BF16, tag="bsb0")
                    a_group(ht0, stage_copy_scalar(bsb0))
                    gsb0 = stg.tile([128, KT * CH], BF16, tag="gsb0")
                    gt_group(ht0, stage_copy_vector(gsb0))
                    ht1, htb1 = load_htc2(1)
                    bsb1 = stg.tile([128, KT * CH], BF16, tag="bsb1")
                    a_group(ht1, stage_copy_scalar(bsb1))
                    gsb1 = stg.tile([128, KT * CH], BF16, tag="gsb1")
                    gt_group(ht1, stage_copy_vector(gsb1))

                    # -- post-collective block --
                    with (
                        tc.tile_pool(name="postsb", bufs=1) as psb,
                        tc.tile_pool(name="postps", bufs=1, space="PSUM") as ps2,
                    ):
                        gb16_s = psb.tile([H, D], F32, tag="gb16")
                        nc.sync.dma_start(out=gb16_s[:], in_=gb16[:])
                        bb16_s = psb.tile([H, D], F32, tag="bb16")
                        nc.sync.dma_start(out=bb16_s[:], in_=bb16[:])
                        Wv_s = ws.tile([128, KT * D], BF16, tag="wstream")
                        for k in range(KT):
                            nc.sync.dma_start(out=Wv_s[:, k * D:(k + 1) * D],
                                              in_=Wv[k * 128:(k + 1) * 128, :])
                        WA_s = ws.tile([128, KT * D], BF16, tag="wstream")
                        for k in range(KT):
                            nc.sync.dma_start(out=WA_s[:, k * D:(k + 1) * D],
                                              in_=WA[k * 128:(k + 1) * 128, :])
                        Gar = psb.tile([H, D], F32, tag="Gar")
                        nc.sync.dma_start(out=Gar[:], in_=arout[:, 0:D])
                        PSar = psb.tile([H, 2], F32, tag="PSar")
                        nc.sync.dma_start(out=PSar[:], in_=arout[:, D:D + 2])
                        sr = psb.tile([H, 1], F32, tag="sr")
                        nc.vector.reciprocal_approx_fast(
                            out=sr[:], in_=PSar[:, 1:2])
                        Gn = psb.tile([H, D], F32, tag="Gn")
                        nc.vector.tensor_scalar_sub(Gn[:], Gar[:],
                                                    PSar[:, 0:1])
                        nc.vector.tensor_mul(Gn[:], Gn[:], gb16_s[:])
                        nc.vector.scalar_tensor_tensor(
                            Gn[:], Gn[:], sr[:, 0:1], bb16_s[:],
                            op0=OP.mult, op1=OP.add)
                        tpg = ps2.tile([128, KT * H], F32, tag="post1")
                        for m in range(KT):
                            nc.tensor.transpose(
                                tpg[:, m * H:(m + 1) * H],
                                Gn[:, m * 128:(m + 1) * 128],
                                idn_s[0:16, 0:16])
                        GnT = psb.tile([128, KT * H], BF16, tag="GnT")
                        nc.vector.tensor_copy(GnT[:], tpg[:])

                        OCp = ps2.tile([128, KT * H], F32, tag="post1")
                        for m in range(KT):
                            for k in range(KT):
                                nc.tensor.matmul(
                                    OCp[:, m * H:(m + 1) * H],
                                    Wv_s[:, k * D + m * 128:k * D + (m + 1) * 128],
                                    GnT[:, k * H:(k + 1) * H],
                                    start=(k == 0), stop=(k == KT - 1))
                        ocv = psb.tile([128, KT], BF16, tag="ocv")
                        for m in range(KT):
                            if m % 2 == 0:
                                nc.vector.tensor_copy(
                                    ocv[0:64, m:m + 1],
                                    OCp[0:64, m * H + 2 * m:m * H + 2 * m + 1])
                                nc.vector.tensor_copy(
                                    ocv[64:128, m:m + 1],
                                    OCp[64:128, m * H + 2 * m + 1:m * H + 2 * m + 2])
                            else:
                                nc.scalar.copy(
                                    ocv[0:64, m:m + 1],
                                    OCp[0:64, m * H + 2 * m:m * H + 2 * m + 1])
                                nc.scalar.copy(
                                    ocv[64:128, m:m + 1],
                                    OCp[64:128, m * H + 2 * m + 1:m * H + 2 * m + 2])

                        a0p = ps2.tile([128, KT], F32, tag="smv")
                        for m in range(KT):
                            for k in range(KT):
                                nc.tensor.matmul(
                                    a0p[:, m:m + 1],
                                    WA_s[:, k * D + m * 128:k * D + (m + 1) * 128],
                                    ocv[:, k:k + 1],
                                    start=(k == 0), stop=(k == KT - 1))
                        nc.vector.scalar_tensor_tensor(
                            a0_s[:], a0p[:], RES, a0c_s[:],
                            op0=OP.mult, op1=OP.add)

                        WG_s = ws.tile([128, KT * D], BF16, tag="wstream")
                        for k in range(KT):
                            nc.sync.dma_start(out=WG_s[:, k * D:(k + 1) * D],
                                              in_=WG[k * 128:(k + 1) * 128, :])
                        g0p = ps2.tile([128, KT], F32, tag="smv")
                        for m in range(KT):
                            for k in range(KT):
                                nc.tensor.matmul(
                                    g0p[:, m:m + 1],
                                    WG_s[:, k * D + m * 128:k * D + (m + 1) * 128],
                                    ocv[:, k:k + 1],
                                    start=(k == 0), stop=(k == KT - 1))
                        nc.vector.scalar_tensor_tensor(
                            g0_s[:], g0p[:], RES, g0c_s[:],
                            op0=OP.mult, op1=OP.add)

                        # h_c_new for the center row (off critical path)
                        Wo_s = ws.tile([128, KT * D], BF16, tag="wstream")
                        for k in range(KT):
                            nc.sync.dma_start(out=Wo_s[:, k * D:(k + 1) * D],
                                              in_=Wo[k * 128:(k + 1) * 128, :])
                        hcp = ps2.tile([128, KT], F32, tag="smv")
                        for m in range(KT):
                            for k in range(KT):
                                nc.tensor.matmul(
                                    hcp[:, m:m + 1],
                                    Wo_s[:, k * D + m * 128:k * D + (m + 1) * 128],
                                    ocv[:, k:k + 1],
                                    start=(k == 0), stop=(k == KT - 1))
                        hcn_sb = psb.tile([128, KT], F32, tag="hcn")
                        nc.vector.scalar_tensor_tensor(
                            hcn_sb[:], hcp[:], RES, hcv_s[:],
                            op0=OP.mult, op1=OP.add)
                        nc.sync.dma_start(out=outC[:], in_=hcn_sb[:])

                    # -- consume staged chunks --
                    for c, (htb, bsb, gsb) in ((0, (htb0, bsb0, gsb0)),
                                               (1, (htb1, bsb1, gsb1))):
                        Bs = sbBs.tile([128, KT, CH], FP8, tag="Bs")
                        fb = silu_into(Bs)
                        for m in range(KT):
                            fb(m, bsb[:, m * CH:(m + 1) * CH])
                        gs = sbBs.tile([128, KT * CH], BF16, tag="gs")
                        fg = sigm_into(gs)
                        for m in range(KT):
                            fg(m, gsb[:, m * CH:(m + 1) * CH])
                        cp_and_out(c, htb, Bs, gs)

                # -- remaining chunks: direct PSUM path --
                for c in range(2, NCH):
                    htc, htb = load_htc2(c)
                    Bs = sbBs.tile([128, KT, CH], FP8, tag="Bs")
                    a_group(htc, silu_into(Bs))
                    gs = sbBs.tile([128, KT * CH], BF16, tag="gs")
                    gt_group(htc, sigm_into(gs))
                    cp_and_out(c, htb, Bs, gs)

            wres_cm.__exit__(None, None, None)
    nc.compile()
    return nc


def _get_nc():
    if "nc" not in _CACHE:
        _CACHE["nc"] = _build()
    return _CACHE["nc"]


def kernel(h, center_idx, rbf_ic, seqsep_ic, nbr_idx, local_bias,
           gamma_c, beta_c, gamma_a, beta_a,
           Wq, Wk, Wv, Wo, Wb, W1, b1, W2, b2, Wg, bg):
    global LAST_RESULTS
    f = np.float32
    bf = ml_dtypes.bfloat16
    h = np.asarray(h, f)
    c = int(center_idx)
    rbf_ic = np.asarray(rbf_ic, f)
    seqsep_ic = np.asarray(seqsep_ic, f)
    nbr_idx = np.asarray(nbr_idx)
    local_bias = np.asarray(local_bias, f)
    gamma_c = np.asarray(gamma_c, np.float64)
    beta_c = np.asarray(beta_c, np.float64)
    gamma_a = np.asarray(gamma_a, np.float64)
    beta_a = np.asarray(beta_a, np.float64)
    Wq = np.asarray(Wq, f); Wk = np.asarray(Wk, f); Wv = np.asarray(Wv, f)
    Wo = np.asarray(Wo, f); Wb = np.asarray(Wb, f)
    W1 = np.asarray(W1, f); b1 = np.asarray(b1, f)
    W2 = np.asarray(W2, f); b2 = np.asarray(b2, f)
    Wg = np.asarray(Wg, f); bg = np.asarray(bg, f)

    # ---- host algebra (tiny, no big matmuls beyond two DxD folds) ----
    f8 = ml_dtypes.float8_e4m3fn
    hc = h[c].astype(np.float64)
    hcl = (hc - hc.mean()) / np.sqrt(hc.var() + EPS) * gamma_c + beta_c
    q = (hcl @ Wq.astype(np.float64)).reshape(H, HD)
    Qm = np.zeros((D, H), np.float64)
    for hh in range(H):
        Qm[hh * HD:(hh + 1) * HD, hh] = q[hh] / np.sqrt(HD)
    Wk1 = Wk.astype(np.float64) @ Qm                    # (D, 16)
    Wkp = (256.0 * Wk1 * gamma_a[:, None]).astype(f8)
    ncg = (-256.0 * (Wk1 * gamma_a[:, None]).sum(0)).astype(f).reshape(H, 1)
    cbv = (Wk1 * beta_a[:, None]).sum(0).astype(f).reshape(H, 1)

    full_bias = np.zeros((N, local_bias.shape[1]), f)
    full_bias[nbr_idx] = local_bias
    bias_featT = np.ascontiguousarray(
        np.concatenate([rbf_ic, seqsep_ic, full_bias], axis=1).T)  # (128, N)

    hT_full8 = np.ascontiguousarray(h.T).astype(f8)     # (D, N) fp8
    hT_fullb = np.ascontiguousarray(h.T).astype(bf)     # (D, N) bf16

    W1b = W1[D:]
    Wgb = Wg[D:]
    WA = (Wo.astype(np.float64) @ W1b.astype(np.float64)).astype(bf)
    WG = (Wo.astype(np.float64) @ Wgb.astype(np.float64)).astype(bf)
    a0c = (h[c].astype(np.float64) @ W1b.astype(np.float64) + b1).astype(f)
    g0c = (h[c].astype(np.float64) @ Wgb.astype(np.float64) + bg).astype(f)

    gamma_a32 = gamma_a.astype(f)
    beta_a32 = beta_a.astype(f)
    Wb_pad = np.zeros((BIAS, 128), f)
    Wb_pad[:, :H] = Wb
    shared = {
        "Wkp8": Wkp, "Wb": Wb_pad.astype(bf),
        "W1t8": np.ascontiguousarray(16.0 * W1[:D]).astype(f8),
        "Wgt8": np.ascontiguousarray(16.0 * Wg[:D]).astype(f8),
        "W2h8": np.ascontiguousarray(16.0 * RES * W2).astype(f8),
        "Wv": Wv.astype(bf), "Wo": Wo.astype(bf),
        "WA": WA, "WG": WG,
        "idn": np.eye(128, dtype=f),
        "ones16": np.ones((1, 128), bf),
        "ncg": ncg, "cbv": cbv,
        "gb16": np.ascontiguousarray(np.broadcast_to(gamma_a32, (H, D))),
        "bb16": np.ascontiguousarray(np.broadcast_to(beta_a32, (H, D))),
        "hcv": np.ascontiguousarray(h[c].reshape(KT, 128).T),
        "a0c": np.ascontiguousarray(a0c.reshape(KT, 128).T),
        "g0c": np.ascontiguousarray(g0c.reshape(KT, 128).T),
        "b2v": np.ascontiguousarray((16.0 * RES * b2).reshape(KT, 128).T),
        "epsv": np.full((NCH, 1), EPS, f),
        "selc8": _selc(),
    }
    in_maps = []
    for i in range(NCORES):
        r0 = i * NS
        m = dict(shared)
        m["hT8"] = np.ascontiguousarray(hT_full8[:, r0:r0 + NS])
        m["hTb"] = np.ascontiguousarray(hT_fullb[:, r0:r0 + NS])
        m["hN8"] = h[r0:r0 + NS].astype(f8)
        m["bT"] = np.ascontiguousarray(bias_featT[:, r0:r0 + NS]).astype(bf)
        in_maps.append(m)

    nc = _get_nc()
    trace = bool(int(os.environ.get("KERNEL_TRACE", "0")))
    res = run_bass_kernel_spmd(nc, in_maps, core_ids=list(range(NCORES)),
                               trace=trace)
    LAST_RESULTS = res

    out = np.empty((N, D), f)
    for i in range(NCORES):
        out[i * NS:(i + 1) * NS] = res.results[i]["outT"].T
    hcn = res.results[0]["outC"].T.reshape(D)           # [m,p] -> flat
    out[c] = hcn
    return out
